# revision 20
# baseline (speedup 1.0000x reference)
"""Trainium2 Bass kernel for DenseIouPred.

The reference op only consumes output[0,0] (4,W,H), target[0,0] (4,),
ind[0,0,0] (scalar) and emits a (W,H) f32 IoU map that is nonzero only
inside a +/-radius window around the center decoded from `ind`.

Evaluated densely over the (h,w) grid the gather+scatter disappears:
  pred(h,w)  = out0[:, h, w]
  t_wl(w)    = t0 + (w - cw)      t_wr(w) = t1 - (w - cw)
  t_ht(h)    = t2 + (h - ch)      t_hb(h) = t3 - (h - ch)
  t_area     = (t0+t1)*(t2+t3)    (constant)
  valid(h,w) = row_mask(w) * col_mask(h)            (separable)
  iou        = (inter+1)/(union+1) where valid else 0

Sharding: the w axis is split across the 8 cores (columns [SH*k, SH*k+SH)
on core k). Each core receives its x column-slice packed as (W, 4*SH) plus
a meta row [t0 t1 t2 t3 ind_bits w_off]; the host concatenates the
per-core (W, SH) outputs. All arithmetic (center decode, box bounds,
masks, IoU) runs on device.

Device notes:
  - center (ch,cw) decoded exactly with a compare-accumulate (no integer
    div/mod on this ISA); all intermediate scalars are exact small ints
  - w-varying terms broadcast across partitions with tiny K=1 bf16 matmuls
    into PSUM (exact: integer-valued); h-varying terms are per-partition
    scalars from a 5-column scalar-broadcast matmul
  - per-engine instruction order is pinned with add_dep_helper so the
    scheduler cannot head-of-line-block the DVE stream; dependent DVE ops
    are kept non-adjacent to hide the same-engine RAW drain latency
  - division is reciprocal+multiply; union+1 is bounded away from 0 on
    this input distribution
"""

import numpy as np

_TRN_REPO = "/opt/trn_rl_repo"


def _ensure_path():
    import sys

    if _TRN_REPO not in sys.path:
        sys.path.insert(0, _TRN_REPO)


_CACHE = {}
N_CORES = 8


def _build(W, H, radius, SH):
    """Bass program for one w-shard: x (W, 4*SH), meta (1,8) -> iou (W, SH)."""
    _ensure_path()
    import concourse.bass as bass
    import concourse.tile as tile
    from concourse.tile import add_dep_helper
    from concourse import mybir

    AOT = mybir.AluOpType
    F32 = mybir.dt.float32
    BF16 = mybir.dt.bfloat16
    I32 = mybir.dt.int32
    R = float(radius)
    assert W == H

    nc = bass.Bass("TRN2", debug=False)
    x_d = nc.dram_tensor("x", [W, 4 * SH], F32, kind="ExternalInput").ap()
    meta_d = nc.dram_tensor("meta", [1, 8], F32, kind="ExternalInput").ap()
    iou_d = nc.dram_tensor("iou", [W, SH], F32, kind="ExternalOutput").ap()

    orders = {"V": [], "G": [], "T": []}

    def _rec(which, inst):
        orders[which].append(inst.ins)
        return inst

    def V(inst):
        return _rec("V", inst)

    def G(inst):
        return _rec("G", inst)

    def T(inst):
        return _rec("T", inst)

    with tile.TileContext(nc) as tc:
        with (
            tc.tile_pool(name="sb", bufs=1) as sb,
            tc.tile_pool(name="ps", bufs=1, space="PSUM") as ps,
        ):
            # ---- loads: meta on SP HWDGE queue, x on ACT HWDGE queue ----
            meta = sb.tile([1, 8], F32)  # [t0 t1 t2 t3 ind_bits w_off 0 0]
            nc.sync.dma_start(meta[:], meta_d[:])
            xt = sb.tile([W, 4 * SH], F32)  # [p_l | p_r | p_t | p_b] slice
            nc.scalar.dma_start(xt[:], x_d[:])
            t0 = meta[0:1, 0:1]
            t1 = meta[0:1, 1:2]
            t2 = meta[0:1, 2:3]
            t3 = meta[0:1, 3:4]
            ind = meta[0:1, 4:5].bitcast(I32)
            woff = meta[0:1, 5:6]  # = W - SH*k  (shard descriptor, host-set)

            # ---- constants (Pool; no input dependency, runs under DMA) ----
            io72 = sb.tile([1, W], I32)  # 0, W, 2W, ...
            G(nc.gpsimd.iota(io72[:], pattern=[[W, W]], base=0,
                             channel_multiplier=0))
            ios = sb.tile([1, SH], I32)  # 0..SH-1
            G(nc.gpsimd.iota(ios[:], pattern=[[1, SH]], base=0,
                             channel_multiplier=0))
            ioc = sb.tile([W, 1], I32)  # partition index column
            G(nc.gpsimd.iota(ioc[:], pattern=[[1, 1]], base=0,
                             channel_multiplier=1))
            ones = sb.tile([1, W], BF16)
            G(nc.gpsimd.memset(ones[:], 1.0))

            # ---- Pool: meta-only scalars (parallel with DVE center chain) ----
            brc = sb.tile([1, 5], BF16)  # [ch, t2, t3, clo, chi]
            tlo = sb.tile([1, 1], F32)
            thi = sb.tile([1, 1], F32)
            G(nc.gpsimd.tensor_copy(brc[0:1, 1:3], meta[0:1, 2:4]))  # t2, t3
            # box+window bounds (exact ints): row valid iff
            # -min(t0,R) <= u <= min(t1,R); col likewise with t2,t3 and v
            G(nc.gpsimd.tensor_scalar(brc[0:1, 3:4], t2, -1.0, -R,
                                      AOT.mult, AOT.max))
            G(nc.gpsimd.tensor_scalar(brc[0:1, 4:5], t3, R, None, AOT.min))
            ts01 = sb.tile([1, 2], F32)
            G(nc.gpsimd.tensor_tensor(ts01[:], meta[0:1, 0:3:2],
                                      meta[0:1, 1:4:2], AOT.add))
            rhs0 = sb.tile([1, 3 * SH + 1], BF16)  # [t_wl|t_wr|row_mask|T+1]
            G(nc.gpsimd.tensor_scalar(rhs0[0:1, 3 * SH : 3 * SH + 1],
                                      ts01[0:1, 0:1], ts01[0:1, 1:2], 1.0,
                                      AOT.mult, AOT.add))

            # ---- PE broadcast targets ----
            S = ps.tile([W, 5], F32)  # [ch, t2, t3, clo, chi] per partition
            P = ps.tile([W, 3 * SH + 1], F32)  # [t_wl_b|t_wr_b|row_mask_b|T1]

            # ---- DVE stream (pinned order; dependent ops non-adjacent) ----
            # center decode: acc = ch+1 (count of W*k <= ind), exact in f32
            q = sb.tile([1, SH], F32)  # ios - ind
            V(nc.vector.tensor_tensor(q[:], ios[:],
                                      ind.broadcast_to([1, SH]), AOT.subtract))
            cmp_t = sb.tile([1, W], F32)
            acc = sb.tile([1, 1], F32)
            V(nc.vector.scalar_tensor_tensor(
                cmp_t[:], ind.broadcast_to([1, W]), 0.0, io72[:],
                AOT.add, AOT.is_ge, accum_out=acc[:]))
            V(nc.vector.tensor_scalar(tlo[:], t0, -1.0, -R, AOT.mult,
                                      AOT.max))  # RAW-gap filler after acc
            accW = sb.tile([1, 1], F32)
            V(nc.vector.tensor_scalar(accW[:], acc[:], float(W), None,
                                      AOT.mult))
            V(nc.vector.tensor_scalar(brc[0:1, 0:1], acc[:], -1.0, None,
                                      AOT.add))  # ch -> brc[0]
            # u = w_global - cw = (ios - ind) + W*(ch+1) - (W - SH*k)
            u = sb.tile([1, SH], F32)
            V(nc.vector.tensor_scalar(u[:], q[:], accW[0:1, 0:1], woff,
                                      AOT.add, AOT.subtract))
            V(nc.vector.tensor_scalar(thi[:], t1, R, None,
                                      AOT.min))  # RAW-gap filler after u
            V(nc.vector.tensor_scalar(rhs0[0:1, 0:SH], u[:], t0, None,
                                      AOT.add))  # t_wl
            m1t = sb.tile([1, SH], F32)
            V(nc.vector.tensor_scalar(m1t[:], u[:], tlo[0:1, 0:1], None,
                                      AOT.is_ge))
            V(nc.vector.tensor_scalar(rhs0[0:1, SH : 2 * SH], u[:], -1.0, t1,
                                      AOT.mult, AOT.add))  # t_wr
            V(nc.vector.scalar_tensor_tensor(
                rhs0[0:1, 2 * SH : 3 * SH], u[:], thi[0:1, 0:1], m1t[:],
                AOT.is_le, AOT.mult))  # row_mask

            # ---- PE broadcasts ----
            T(nc.tensor.matmul(S[:], ones[:], brc[:], start=True, stop=True))
            T(nc.tensor.matmul(P[:, 0 : 2 * SH], ones[:],
                               rhs0[0:1, 0 : 2 * SH], start=True, stop=True))
            T(nc.tensor.matmul(P[:, 2 * SH : 3 * SH + 1], ones[:],
                               rhs0[0:1, 2 * SH : 3 * SH + 1],
                               start=True, stop=True))

            # ---- Pool: p_area pipeline (SBUF-only; Pool must not touch PSUM) ----
            AB = sb.tile([W, 2 * SH], F32)  # [a+b | c+d]
            x_r = xt[:].rearrange("h (i j w) -> h i j w", i=2, j=2)
            G(nc.gpsimd.tensor_tensor(
                AB[:].rearrange("h (i w) -> h i w", i=2),
                x_r[:, :, 0, :], x_r[:, :, 1, :], AOT.add))
            PA = sb.tile([W, SH], F32)
            G(nc.gpsimd.tensor_tensor(PA[:], AB[:, 0:SH], AB[:, SH : 2 * SH],
                                      AOT.mult))

            # ---- DVE columns + dense, interleaved to hide RAW latency ----
            Sc = sb.tile([W, 5], F32)  # S staged to SBUF (cheaper DVE reads)
            V(nc.vector.tensor_copy(Sc[:], S[:]))
            v_c = sb.tile([W, 1], F32)  # h - ch
            V(nc.vector.tensor_scalar(v_c[:], ioc[:], Sc[:, 0:1], None,
                                      AOT.subtract))
            tht_c = sb.tile([W, 1], F32)
            V(nc.vector.tensor_tensor(tht_c[:], v_c[:], Sc[:, 1:2], AOT.add))
            thb_c = sb.tile([W, 1], F32)
            V(nc.vector.scalar_tensor_tensor(thb_c[:], v_c[:], -1.0,
                                             Sc[:, 2:3], AOT.mult, AOT.add))
            c1 = sb.tile([W, 1], F32)
            V(nc.vector.tensor_tensor(c1[:], v_c[:], Sc[:, 3:4], AOT.is_ge))
            cm_c = sb.tile([W, 1], F32)
            V(nc.vector.scalar_tensor_tensor(cm_c[:], v_c[:], Sc[:, 4:5],
                                             c1[:], AOT.is_le, AOT.mult))
            min2 = sb.tile([W, 2 * SH], F32)  # [min(a,t_wl) | min(b,t_wr)]
            V(nc.vector.tensor_tensor(min2[:], xt[:, 0 : 2 * SH],
                                      P[:, 0 : 2 * SH], AOT.min))
            md = sb.tile([W, SH], F32)  # min(p_b, t_hb)
            V(nc.vector.tensor_scalar(md[:], xt[:, 3 * SH : 4 * SH],
                                      thb_c[:, 0:1], None, AOT.min))
            w_int = sb.tile([W, SH], F32)
            V(nc.vector.tensor_tensor(w_int[:], min2[:, 0:SH],
                                      min2[:, SH : 2 * SH], AOT.add))
            h_int = sb.tile([W, SH], F32)  # min(p_t, t_ht) + md
            V(nc.vector.scalar_tensor_tensor(
                h_int[:], xt[:, 2 * SH : 3 * SH], tht_c[:, 0:1], md[:],
                AOT.min, AOT.add))
            M = sb.tile([W, SH], F32)  # row_mask_b * col_mask
            V(nc.vector.tensor_scalar(M[:], P[:, 2 * SH : 3 * SH],
                                      cm_c[:, 0:1], None, AOT.mult))
            inter = sb.tile([W, SH], F32)
            V(nc.vector.tensor_tensor(inter[:], w_int[:], h_int[:], AOT.mult))
            U1 = sb.tile([W, SH], F32)  # union+1 = (p_area + (T+1)) - inter
            V(nc.vector.scalar_tensor_tensor(
                U1[:], PA[:], P[:, 3 * SH : 3 * SH + 1], inter[:],
                AOT.add, AOT.subtract))
            NM = sb.tile([W, SH], F32)  # (inter+1) * M
            V(nc.vector.scalar_tensor_tensor(NM[:], inter[:], 1.0, M[:],
                                             AOT.add, AOT.mult))
            REC = sb.tile([W, SH], F32)
            V(nc.vector.reciprocal(REC[:], U1[:]))
            RES = sb.tile([W, SH], F32)
            V(nc.vector.tensor_tensor(RES[:], NM[:], REC[:], AOT.mult))
            nc.sync.dma_start(iou_d[:], RES[:])

            # pin per-engine program order so the scheduler cannot reorder
            # streams into head-of-line blocking
            for seq in orders.values():
                for a, b in zip(seq[1:], seq[:-1]):
                    add_dep_helper(a, b, sync=False, reason="pinned stream order")

    _postprocess(nc)
    return nc


_SPLIT_N = [0]


def _postprocess(nc):
    """(1) This walrus build only supports one sync-wait per instruction;
    hoist extra waits into standalone NoOps on the same engine, placed
    before. (2) Drop the dead const-* preamble memsets (no readers here)
    and the preamble's head all-engine barrier (cross-engine deps are all
    carried by tile semaphores; the tail still double-barriers, which is
    what guards re-execution)."""
    _ensure_path()
    from concourse import mybir

    for f in nc.m.functions:
        for b in f.blocks:
            insts = b.instructions
            new = []
            changed = False
            for inst in insts:
                if b.name == "main" and isinstance(
                    inst, mybir.InstDrain | mybir.InstEventSemaphore
                ):
                    changed = True
                    continue
                if (
                    isinstance(inst, mybir.InstMemset)
                    and inst.outs
                    and getattr(inst.outs[0], "memref", "").startswith("const-")
                    and not (inst.sync_info and (inst.sync_info.on_wait
                                                 or inst.sync_info.on_update))
                ):
                    changed = True
                    continue
                si = inst.sync_info
                if si is not None and si.on_wait and len(si.on_wait) > 1:
                    waits = list(si.on_wait)
                    for w in waits[:-1]:
                        _SPLIT_N[0] += 1
                        n = mybir.InstNoOp(name=f"splitwait-{_SPLIT_N[0]}")
                        n.engine = inst.engine
                        n.sync_info = mybir.SyncInfo(on_wait=[w], on_update=[])
                        new.append(n)
                    si.on_wait = waits[-1:]
                    changed = True
                new.append(inst)
            if changed:
                b.instructions = new


def _get_program(W, H, radius, SH):
    key = (W, H, int(radius), SH)
    if key not in _CACHE:
        _CACHE[key] = _build(W, H, radius, SH)
    return _CACHE[key]


def _pack_inputs(output, ind, target):
    output = np.asarray(output)
    W, H = output.shape[-2], output.shape[-1]
    dim = output.shape[-3] if output.ndim >= 3 else 4
    SH = H // N_CORES
    out0 = output.reshape(-1, dim, W, H)[0]
    xhcw = np.ascontiguousarray(
        out0.transpose(1, 0, 2), dtype=np.float32
    )  # (W, dim, H): [h, c, w]
    tgt = np.asarray(target, dtype=np.float32).reshape(-1, dim)[0]
    ind0 = np.int32(np.asarray(ind).reshape(-1)[0])
    ind_bits = np.array([ind0], dtype=np.int32).view(np.float32)[0]
    in_maps = []
    for k in range(N_CORES):
        xk = np.ascontiguousarray(
            xhcw[:, :, SH * k : SH * (k + 1)]
        ).reshape(W, dim * SH)
        meta = np.zeros((1, 8), dtype=np.float32)
        meta[0, 0:4] = tgt
        meta[0, 4] = ind_bits
        meta[0, 5] = float(W - SH * k)
        in_maps.append({"x": xk, "meta": meta})
    return W, H, SH, in_maps


def kernel(output, ind, target, radius):
    _ensure_path()
    from concourse.bass_utils import run_bass_kernel_spmd

    W, H, SH, in_maps = _pack_inputs(output, ind, target)
    nc = _get_program(W, H, int(radius), SH)
    res = run_bass_kernel_spmd(nc, in_maps, core_ids=list(range(N_CORES)))
    return np.concatenate([r["iou"] for r in res.results], axis=1)
